# revision 1
# baseline (speedup 1.0000x reference)
# Trainium2 Bass kernel for nn_DifferentiableForest.
#
# Math (per batch row b):
#   dec[t,n]   = sigmoid(x @ Wd[t,n] + bd[t,n])           t<10 trees, n<15 nodes
#   path[t,l]  = torch-style tile/interleave product of (dec, 1-dec) over 4 levels
#   leaf[t,l,c]= softmax_c(x @ Wl[t,l,c] + bl[t,l,c])     l<16 leaves, c<10 classes
#   out[b,c]   = sum_t softmax(tree_w)[t] * sum_l path * leaf
#
# Device mapping (pure data parallel over 8 cores, batch-sharded):
#   - x is cast to bf16 and transposed host-side; weights are reordered and
#     cast to bf16 host-side (softmax(tree_w)/16 folded into the path product).
#   - PE: z = x @ W for all decision nodes + all leaf logits (bf16, fp32 psum);
#         biases enter via a K=1 matmul against a ones stationary.
#   - ACT: h = tanh(z/2) (sigmoid substitute, same table set as exp), E=exp(z).
#   - DVE: path product (strided TT), S = per-(t,l) softmax denominators
#          (pair-add tree over the 10 class blocks), 1/S (fast reciprocal),
#          q = path/S, G = E*q, and per-class accumulation via tensor_scalar
#          accum_out (fp32).
# Leaf layout is class-major (c, t, l) so the c-broadcast of q and the final
# per-class reductions are contiguous; the leaf index l is stored in the
# kernel's "sign-block" order with ref_leaf = 2*(l & 7) + (l >> 3).

import numpy as np
import ml_dtypes

import concourse.bass as bass
import concourse.mybir as mybir
import concourse.tile as tile
from concourse.bass_utils import run_bass_kernel_spmd

BF16 = mybir.dt.bfloat16
F32 = mybir.dt.float32
AL = mybir.AluOpType

B, F, C, T, D = 131072, 256, 10, 10, 4
ND, L = 2 ** D - 1, 2 ** D          # 15, 16
NL = C * T * L                      # 1600 leaf logits / row
NDK = T * ND                        # 150 decision nodes / row
NCORES = 8
BC = B // NCORES                    # 16384 rows per core
NTILES = BC // 128                  # 128
import os
GRP = int(os.environ.get('K_GRP', '4'))   # row-tiles per DVE batch group
NGRP = NTILES // GRP
CHUNKS = [(0, 512), (512, 512), (1024, 512), (1536, 64)]  # psum-bank slices of NL
LV_OFF = [0, 10, 30, 70]            # level-major offsets into the 150 dec nodes


def _ap(base, extra_off, dims):
    """AP with base's partition dim, custom free dims [[step,count],...] (elements)."""
    return bass.AP(tensor=base.tensor, offset=base.offset + extra_off, ap=[base.ap[0]] + dims)


def _split_excess_waits(nc, max_waits=1):
    # This walrus rejects CTRL-class instructions (Drain/EventSemaphore) with
    # more than one sem wait; move extras onto same-engine NoOps placed before.
    n = 0
    for f in nc.m.functions:
        for bb in f.blocks:
            out, changed = [], False
            for ins in bb.instructions:
                si = ins.sync_info
                ow = list(si.on_wait) if si is not None else []
                if len(ow) > max_waits:
                    for wv in ow[:-max_waits]:
                        nop = mybir.InstNoOp(name=f"wsplit-{n}", ins=[], outs=[])
                        nop.engine = ins.engine
                        nop.sync_info = mybir.SyncInfo(on_wait=[wv], on_update=[])
                        out.append(nop)
                        n += 1
                    si.on_wait = ow[-max_waits:]
                    ins.sync_info = si
                    changed = True
                out.append(ins)
            if changed:
                bb.instructions = out
    return n


def _build_program():
    nc = bass.Bass()
    xT = nc.dram_tensor("xT", [2, 128, BC], BF16, kind="ExternalInput")
    Wl_d = nc.dram_tensor("Wl", [2, 128, NL], BF16, kind="ExternalInput")
    Wd_d = nc.dram_tensor("Wd", [2, 128, NDK], BF16, kind="ExternalInput")
    blr_d = nc.dram_tensor("blr", [1, NL], BF16, kind="ExternalInput")
    bdr_d = nc.dram_tensor("bdr", [1, NDK], BF16, kind="ExternalInput")
    w16_d = nc.dram_tensor("w16", [1, T], BF16, kind="ExternalInput")
    y = nc.dram_tensor("y", [BC, C], F32, kind="ExternalOutput")

    with tile.TileContext(nc) as tc:
        with (
            tc.tile_pool(name="persist", bufs=1) as persist,
            tc.tile_pool(name="psum", bufs=2, space="PSUM") as psum,
            tc.tile_pool(name="eg", bufs=int(os.environ.get("K_EGBUFS", "2"))) as egp,
            tc.tile_pool(name="gg", bufs=int(os.environ.get("K_GGBUFS", "2"))) as ggp,
            tc.tile_pool(name="small", bufs=int(os.environ.get("K_SBUFS", "2"))) as sp,
            tc.tile_pool(name="hh", bufs=int(os.environ.get("K_HHBUFS", "2"))) as hhp,
            tc.tile_pool(name="dump", bufs=int(os.environ.get("K_DUMPBUFS", "2"))) as dpp,
            tc.tile_pool(name="outp", bufs=2) as outp,
        ):
            # ---- persistent loads ----
            xT_sb = persist.tile([128, 2, BC], BF16)
            for k in range(2):
                for c0 in range(0, BC, 4096):
                    nc.sync.dma_start(xT_sb[:, k, c0:c0 + 4096], xT[k, :, c0:c0 + 4096])
            Wl_sb = persist.tile([128, 2, NL], BF16)
            Wd_sb = persist.tile([128, 2, NDK], BF16)
            for k in range(2):
                nc.sync.dma_start(Wl_sb[:, k, :], Wl_d[k])
                nc.sync.dma_start(Wd_sb[:, k, :], Wd_d[k])
            blr_sb = persist.tile([1, NL], BF16)
            bdr_sb = persist.tile([1, NDK], BF16)
            nc.sync.dma_start(blr_sb[:], blr_d[:])
            nc.sync.dma_start(bdr_sb[:], bdr_d[:])
            ones_sb = persist.tile([1, 128], BF16)
            nc.vector.memset(ones_sb[:], 1.0)
            w16_sb = persist.tile([128, T], BF16)
            w16_bcast = bass.AP(tensor=w16_d, offset=0, ap=[[0, 128], [1, T]])
            nc.gpsimd.dma_start(w16_sb[:], w16_bcast)

            for g in range(NGRP):
                # ---------- phase 1: decision nodes ----------
                HH = hhp.tile([128, GRP, NDK], BF16, tag="HH")
                UU = sp.tile([128, GRP, 2 * NDK], BF16, tag="UU")
                for j in range(GRP):
                    ti = g * GRP + j
                    cs = ti * 128
                    pd = psum.tile([128, NL], F32, tag="ps")
                    nc.tensor.matmul(pd[:, :NDK], ones_sb[:], bdr_sb[:], start=True, stop=False)
                    nc.tensor.matmul(pd[:, :NDK], xT_sb[:, 0, cs:cs + 128], Wd_sb[:, 0, :], start=False, stop=False)
                    nc.tensor.matmul(pd[:, :NDK], xT_sb[:, 1, cs:cs + 128], Wd_sb[:, 1, :], start=False, stop=True)
                    nc.scalar.activation(HH[:, j, :], pd[:, :NDK], mybir.ActivationFunctionType.Tanh, scale=0.5)
                # u+ = 1 + h ; u- = 1 - h   (group-batched)
                nc.vector.tensor_scalar(UU[:, :, 0:NDK], HH[:], 1.0, None, AL.add)
                nc.vector.tensor_scalar(UU[:, :, NDK:2 * NDK], HH[:], -1.0, 1.0, AL.mult, AL.add)
                # path product, sign-block layout; w/16 folded at level 0
                UUb = UU[:]
                PB = [sp.tile([128, GRP, T * 2 ** (d + 1)], BF16, tag=f"PB{d}", name=f"PB{d}") for d in range(D)]
                # level 0: out[g, t, s] = w16[t] * u_s[t]
                nc.vector.tensor_tensor(
                    _ap(PB[0][:], 0, [[2 * T, GRP], [2, T], [1, 2]]),
                    _ap(w16_sb[:], 0, [[0, GRP], [1, T], [0, 2]]),
                    _ap(UUb, 0, [[2 * NDK, GRP], [1, T], [NDK, 2]]),
                    AL.mult,
                )
                for d in range(1, D):
                    half = 2 ** (d - 1)
                    szin, szout = T * 2 ** d, T * 2 ** (d + 1)
                    for s in range(2):
                        # out[g,t,r,n'] = prev[g,t,(n mod half)] * u_s[lvl d][g,t,n],  n = r*half+n'
                        nc.vector.tensor_tensor(
                            _ap(PB[d][:], s * 2 ** d, [[szout, GRP], [2 ** (d + 1), T], [half, 2], [1, half]]),
                            _ap(PB[d - 1][:], s * half, [[szin, GRP], [2 ** d, T], [0, 2], [1, half]]),
                            _ap(UUb, LV_OFF[d] + s * NDK, [[2 * NDK, GRP], [2 ** d, T], [half, 2], [1, half]]),
                            AL.mult,
                        )
                PATH = PB[D - 1]  # [128, GRP, 160]

                # ---------- phase 2: leaves ----------
                EG = egp.tile([128, GRP, NL], BF16, tag="EG")
                for j in range(GRP):
                    ti = g * GRP + j
                    cs = ti * 128
                    pl = psum.tile([128, NL], F32, tag="ps")
                    for c0, n in CHUNKS:
                        nc.tensor.matmul(pl[:, c0:c0 + n], ones_sb[:], blr_sb[:, c0:c0 + n], start=True, stop=False)
                    for k in range(2):
                        for c0, n in CHUNKS:
                            nc.tensor.matmul(pl[:, c0:c0 + n], xT_sb[:, k, cs:cs + 128], Wl_sb[:, k, c0:c0 + n],
                                             start=False, stop=(k == 1))
                    nc.scalar.activation(EG[:, j, :], pl[:], mybir.ActivationFunctionType.Exp)

                # S = sum over the 10 class blocks (pair-add tree, last level fp32)
                S1 = sp.tile([128, GRP, 800], BF16, tag="S1")
                S2 = sp.tile([128, GRP, 320], BF16, tag="S2")
                S3 = sp.tile([128, GRP, 160], BF16, tag="S3")
                SF = sp.tile([128, GRP, 160], F32, tag="SF")
                e4 = EG[:].rearrange("p g (b x) -> p g b x", b=C)
                s14 = S1[:].rearrange("p g (b x) -> p g b x", b=5)
                s24 = S2[:].rearrange("p g (b x) -> p g b x", b=2)
                nc.vector.tensor_tensor(s14, e4[:, :, 0:10:2, :], e4[:, :, 1:10:2, :], AL.add)
                nc.vector.tensor_tensor(s24, s14[:, :, 0:4:2, :], s14[:, :, 1:4:2, :], AL.add)
                nc.vector.tensor_tensor(S3[:], S2[:, :, 0:160], S2[:, :, 160:320], AL.add)
                nc.vector.tensor_tensor(SF[:], S3[:], s14[:, :, 4, :], AL.add)
                IV = sp.tile([128, GRP, 160], F32, tag="IV")
                IVB = sp.tile([128, GRP, 160], BF16, tag="IVB")
                QQ = sp.tile([128, GRP, 160], BF16, tag="QQ")
                nc.vector.reciprocal(IV[:], SF[:])
                nc.vector.tensor_copy(IVB[:], IV[:])
                nc.vector.tensor_tensor(QQ[:], IVB[:], PATH[:], AL.mult)

                # G = E * q (q broadcast over the class dim)
                GG = ggp.tile([128, GRP, NL], BF16, tag="GG")
                nc.vector.tensor_tensor(
                    GG[:].rearrange("p g (b x) -> p g b x", b=C),
                    e4,
                    _ap(QQ[:], 0, [[160, GRP], [0, C], [1, 160]]),
                    AL.mult,
                )
                # out[c] = fp32 accumulation of each class block
                OG = outp.tile([128, GRP, C], F32, tag="OG")
                dump = dpp.tile([128, 160], BF16, tag="dump")
                for j in range(GRP):
                    for c in range(C):
                        nc.vector.tensor_scalar(
                            dump[:], GG[:, j, c * 160:(c + 1) * 160], 1.0, None, AL.mult, AL.add,
                            accum_out=OG[:, j, c:c + 1],
                        )
                for j in range(GRP):
                    ti = g * GRP + j
                    nc.sync.dma_start(y[ti * 128:(ti + 1) * 128, :], OG[:, j, :])

    _split_excess_waits(nc)
    nc.finalize()
    return nc


_NC = None


def _prep_weights(Wd, bd, Wl, bl, tree_w):
    bf = ml_dtypes.bfloat16
    tw = np.asarray(tree_w, np.float64)
    w = np.exp(tw - tw.max())
    w = (w / w.sum()).astype(np.float32)
    lv_sl = [(2 ** d - 1, 2 ** (d + 1) - 1) for d in range(D)]
    Wd_cols = np.concatenate([np.asarray(Wd)[:, s:e, :].reshape(T * (e - s), F) for s, e in lv_sl], 0)
    bd_cols = np.concatenate([np.asarray(bd)[:, s:e].reshape(-1) for s, e in lv_sl], 0)
    perm = np.array([2 * (m & 7) + (m >> 3) for m in range(L)])
    Wl_cols = np.transpose(np.asarray(Wl)[:, perm], (2, 0, 1, 3)).reshape(NL, F)
    bl_cols = np.transpose(np.asarray(bl)[:, perm], (2, 0, 1)).reshape(NL)
    WdT = np.ascontiguousarray(Wd_cols.T.astype(bf)).reshape(2, 128, NDK)
    WlT = np.ascontiguousarray(Wl_cols.T.astype(bf)).reshape(2, 128, NL)
    return {
        "Wl": WlT,
        "Wd": WdT,
        "blr": bl_cols.astype(bf).reshape(1, NL),
        "bdr": bd_cols.astype(bf).reshape(1, NDK),
        "w16": (w / 16.0).astype(bf).reshape(1, T),
    }


def kernel(x, Wd, bd, Wl, bl, tree_w):
    global _NC
    if _NC is None:
        _NC = _build_program()
    shared = _prep_weights(Wd, bd, Wl, bl, tree_w)
    xT_all = np.ascontiguousarray(np.asarray(x).T.astype(ml_dtypes.bfloat16))  # [F, B]
    in_maps = []
    for c in range(NCORES):
        xc = np.ascontiguousarray(xT_all[:, c * BC:(c + 1) * BC]).reshape(2, 128, BC)
        m = {"xT": xc}
        m.update(shared)
        in_maps.append(m)
    res = run_bass_kernel_spmd(_NC, in_maps, core_ids=list(range(NCORES)))
    return np.concatenate([r["y"] for r in res.results], axis=0)

